# revision 2
# baseline (speedup 1.0000x reference)
"""Trainium2 Bass kernel for nn_Encoder (DA-RNN style input-attention LSTM encoder).

Math insight: the per-batch scalar (h@w_h + c@w_c + b_attn) added to the
attention logits is constant along the softmax axis, so
    attn = softmax(einsum('btd,t->bd', x, w_x))      (recurrence-independent)
    input_weighted[b,t,:] = attn[b,:] * x[b,t,:]
and the recurrence is a plain LSTM over wx with per-step gates
    gates = wx_t @ w_ih.T + h @ w_hh.T + (b_ih + b_hh).

Sharding: data-parallel, 128 batch rows per core x 8 cores, weights replicated.

Per-core layout: batch (128) on SBUF partitions. Per step the PE accumulates
gates[128, 2048] in PSUM from float32r matmuls:
  bias (rank-1 ones x brow), 4 k-chunks of wxT_t @ W_ih, 4 k-chunks of
  hT_{t-1} @ W_hh.  Stationaries are the transposed activations (PE transpose
  + ACT copy to f32r); moving operand is the combined pre-transposed weight.
Issue order software-pipelines: A-mms(t) | tr_h(t-1) | wx-production(t+2) |
B-mms(t) | elementwise(t), so PE stays busy through the elementwise chain.
"""
import numpy as np
from contextlib import ExitStack

import concourse.bass as bass
import concourse.tile as tile
from concourse import bacc, mybir
from concourse.bass_utils import run_bass_kernel_spmd

F32 = mybir.dt.float32
F32R = mybir.dt.float32r
AF = mybir.ActivationFunctionType
X = mybir.AxisListType.X

B, T, D, H = 1024, 64, 512, 512
NCORES = 8
BLOC = B // NCORES          # 128 = partition count
H4 = 4 * H                  # 2048
LA = 2                      # wx production lookahead (steps)
NORD = [1, 2, 0, 3]         # n-chunk (gate) order: f, g, i, o

_NC_CACHE = {}


def build(t_steps=T):
    nc = bacc.Bacc(None)
    x_d = nc.declare_dram_parameter("x", [BLOC, T, D], F32R, isOutput=False)
    w_d = nc.declare_dram_parameter("wcomb", [128, 8, H4], F32R, isOutput=False)
    br_d = nc.declare_dram_parameter("brow", [1, H4], F32R, isOutput=False)
    wxid_d = nc.declare_dram_parameter("wxid", [128, T, 128], F32R, isOutput=False)
    id_d = nc.declare_dram_parameter("ident", [128, 128], F32, isOutput=False)
    ones_d = nc.declare_dram_parameter("ones", [1, 128], F32R, isOutput=False)
    ow_d = nc.declare_dram_parameter("out_w", [BLOC, T, D], F32, isOutput=True)
    oh_d = nc.declare_dram_parameter("out_h", [BLOC, T, H], F32, isOutput=True)

    ts = bass.ts
    with tile.TileContext(nc) as tc, ExitStack() as ctx:
        const = ctx.enter_context(tc.tile_pool(name="const", bufs=1))
        main = ctx.enter_context(tc.tile_pool(name="main", bufs=1))
        xp = ctx.enter_context(tc.tile_pool(name="xp", bufs=6))

        W = const.tile([128, 8, H4], F32R)
        nc.sync.dma_start(W[:], w_d[:])
        brow = const.tile([1, H4], F32R)
        nc.sync.dma_start(brow[:], br_d[:])
        ident = const.tile([128, 128], F32)
        nc.sync.dma_start(ident[:], id_d[:])
        ones1 = const.tile([1, 128], F32R)
        nc.sync.dma_start(ones1[:], ones_d[:])
        # dependency-free dummy to absorb the Exp ACT table load early
        dmy = const.tile([128, 1], F32)
        nc.gpsimd.memset(dmy[:], 0.0)
        dmy2 = const.tile([128, 1], F32)
        nc.scalar.activation(dmy2[:], dmy[:], AF.Exp)

        # ---- preamble: logit_x accumulation + softmax -> attn ----
        attn = main.tile([128, D], F32)
        with (
            tc.tile_pool(name="pre", bufs=1) as pre,
            tc.tile_pool(name="prepsum", bufs=1, space=bass.MemorySpace.PSUM) as prepsum,
        ):
            wxid = pre.tile([128, T, 128], F32R)
            nc.sync.dma_start(wxid[:], wxid_d[:])
            plog = prepsum.tile([128, D], F32)
            for t in range(T):
                xt = xp.tile([128, D], F32R, tag="x")
                nc.sync.dma_start(xt[:], x_d[:, t, :])
                nc.tensor.matmul(plog[:], wxid[:, t, :], xt[:],
                                 start=(t == 0), stop=(t == T - 1))
            negmax = pre.tile([128, 1], F32)
            nc.vector.reduce_max(negmax[:], plog[:], axis=X, negate=True)
            e = pre.tile([128, D], F32)
            nc.scalar.activation(e[:], plog[:], AF.Exp, bias=negmax[:])
            ssum = pre.tile([128, 1], F32)
            nc.vector.reduce_sum(ssum[:], e[:], axis=X)
            rinv = pre.tile([128, 1], F32)
            nc.vector.reciprocal(rinv[:], ssum[:])
            nc.vector.tensor_scalar_mul(attn[:], e[:], rinv[:])
        # switch ACT tables to the sigmoid/tanh set off the critical path;
        # input depends on attn so the scheduler keeps it after the Exp.
        nc.scalar.activation(dmy2[:], attn[:, 0:1], AF.Sigmoid)

        # ---- main-loop pools (PSUM budget: 6 gates + 1 trw + 1 trh = 8 banks) ----
        wxp = ctx.enter_context(tc.tile_pool(name="wxp", bufs=3))
        wxtp = ctx.enter_context(tc.tile_pool(name="wxtp", bufs=4))
        sig = ctx.enter_context(tc.tile_pool(name="sig", bufs=2))
        state = ctx.enter_context(tc.tile_pool(name="state", bufs=2))
        gps = ctx.enter_context(
            tc.tile_pool(name="gps", bufs=6, space=bass.MemorySpace.PSUM))
        trp = ctx.enter_context(
            tc.tile_pool(name="trp", bufs=1, space=bass.MemorySpace.PSUM))

        def produce_wx(t):
            xt = xp.tile([128, D], F32R, tag="x")
            nc.sync.dma_start(xt[:], x_d[:, t, :])
            wx = wxp.tile([128, D], F32, tag="wx")
            nc.vector.tensor_mul(wx[:], attn[:], xt[:].bitcast(F32))
            nc.sync.dma_start(ow_d[:, t, :], wx[:])
            ptr = trp.tile([128, D], F32, tag="trw")
            for c in range(4):
                nc.tensor.transpose(ptr[:, ts(c, 128)], wx[:, ts(c, 128)], ident[:])
            wxT = wxtp.tile([128, D], F32R, tag="wxT")
            nc.scalar.copy(wxT[:], ptr[:])
            return wxT

        wxTs = {}
        for t in range(min(LA, t_steps)):
            wxTs[t] = produce_wx(t)

        c_prev = state.tile([128, H], F32, tag="c")
        nc.gpsimd.memset(c_prev[:], 0.0)
        h_prev = None

        for t in range(t_steps):
            wxT = wxTs.pop(t)
            # A: bias + wx-part matmuls (independent of recurrence)
            g_ps = {}
            for n in NORD:
                gp = gps.tile([128, 512], F32, tag="gates")
                g_ps[n] = gp
                nc.tensor.matmul(gp[:], ones1[:], brow[0:1, ts(n, 512)],
                                 start=True, stop=False)
                for k in range(4):
                    nc.tensor.matmul(gp[:], wxT[:, ts(k, 128)], W[:, k, ts(n, 512)],
                                     start=False, stop=(t == 0 and k == 3))
            # transpose h_{t-1} (PE) + copy to f32r SBUF (ACT)
            if t >= 1:
                ptrh = trp.tile([128, H], F32, tag="trh")
                for c in range(4):
                    nc.tensor.transpose(ptrh[:, ts(c, 128)], h_prev[:, ts(c, 128)],
                                        ident[:])
                hT = state.tile([128, H], F32R, tag="hT")
                nc.scalar.copy(hT[:], ptrh[:])
            # wx production for t+LA (DVE mul + PE transposes, fills PE queue)
            if t + LA < t_steps:
                wxTs[t + LA] = produce_wx(t + LA)
            # B: h-part matmuls
            if t >= 1:
                for n in NORD:
                    for k in range(4):
                        nc.tensor.matmul(g_ps[n][:], hT[:, ts(k, 128)],
                                         W[:, 4 + k, ts(n, 512)],
                                         start=False, stop=(k == 3))
            # elementwise LSTM cell (banks complete in order f, g, i, o)
            sf = sig.tile([128, 512], F32, tag="sf")
            nc.scalar.activation(sf[:], g_ps[1][:], AF.Sigmoid)
            tg = sig.tile([128, 512], F32, tag="tg")
            nc.scalar.activation(tg[:], g_ps[2][:], AF.Tanh)
            t1 = sig.tile([128, 512], F32, tag="t1")
            nc.vector.tensor_mul(t1[:], sf[:], c_prev[:])
            si = sig.tile([128, 512], F32, tag="si")
            nc.scalar.activation(si[:], g_ps[0][:], AF.Sigmoid)
            t2 = sig.tile([128, 512], F32, tag="t2")
            nc.vector.tensor_mul(t2[:], si[:], tg[:])
            c_new = state.tile([128, H], F32, tag="c")
            nc.vector.tensor_add(c_new[:], t1[:], t2[:])
            so = sig.tile([128, 512], F32, tag="so")
            nc.scalar.activation(so[:], g_ps[3][:], AF.Sigmoid)
            tcc = sig.tile([128, 512], F32, tag="tcc")
            nc.scalar.activation(tcc[:], c_new[:], AF.Tanh)
            h = state.tile([128, H], F32, tag="h")
            nc.vector.tensor_mul(h[:], so[:], tcc[:])
            nc.sync.dma_start(oh_d[:, t, :], h[:])
            c_prev = c_new
            h_prev = h

    nc.compile()
    return nc


def _host_prep(input_data, w_ih, w_hh, b_ih, b_hh, w_attn, b_attn):
    w_x = np.ascontiguousarray(w_attn[0, 2 * H:]).astype(np.float32)   # [T]
    # wcomb[p, c, n] = concat(w_ih, w_hh, axis=1).T[c*128+p, n]
    wc = np.concatenate([w_ih, w_hh], axis=1).T.astype(np.float32)     # [1024, 2048]
    wcomb = np.ascontiguousarray(wc.reshape(8, 128, H4).transpose(1, 0, 2))
    brow = np.ascontiguousarray((b_ih + b_hh)[None, :]).astype(np.float32)
    # wxid[p, t, m] = w_x[t] * eye(128)[p, m]
    eye = np.eye(128, dtype=np.float32)
    wxid = np.ascontiguousarray((w_x[None, :, None] * eye[:, None, :]))
    ident = eye.copy()
    ones = np.ones((1, 128), dtype=np.float32)
    return wcomb, brow, wxid, ident, ones


def kernel(input_data, w_ih, w_hh, b_ih, b_hh, w_attn, b_attn):
    input_data = np.asarray(input_data, dtype=np.float32)
    wcomb, brow, wxid, ident, ones = _host_prep(
        np.asarray(input_data), np.asarray(w_ih, np.float32),
        np.asarray(w_hh, np.float32), np.asarray(b_ih, np.float32),
        np.asarray(b_hh, np.float32), np.asarray(w_attn, np.float32),
        np.asarray(b_attn, np.float32))

    if "nc" not in _NC_CACHE:
        _NC_CACHE["nc"] = build()
    nc = _NC_CACHE["nc"]

    in_maps = []
    for c in range(NCORES):
        shard = np.ascontiguousarray(input_data[c * BLOC:(c + 1) * BLOC])
        in_maps.append({"x": shard, "wcomb": wcomb, "brow": brow,
                        "wxid": wxid, "ident": ident, "ones": ones})
    res = run_bass_kernel_spmd(nc, in_maps, list(range(NCORES)))
    out_w = np.concatenate([r["out_w"] for r in res.results], axis=0)
    out_h = np.concatenate([r["out_h"] for r in res.results], axis=0)
    return out_w, out_h
